# revision 1
# baseline (speedup 1.0000x reference)
"""ChineseCLIPVisionLayer on 8 trn2 NeuronCores.

Sharding: pure data-parallel over batch (B=32 -> 4 per core), zero
collectives. Weights are host-transposed and replicated to every core.

Per-core pipeline (all activations that feed matmuls live in transposed
layout [D, S] so the contraction dim sits on SBUF partitions):
  LN1 (natural) -> PE-transpose -> h^T
  q^T,k^T (transposed out), v (natural out)
  per head: scores = q_h^T.T @ k_h^T -> softmax (no max-sub; scores ~ +-3)
            probs -> PE-transpose -> probs^T
            attn^T = v_h.T @ probs^T   (+v_b via bias: softmax rows sum to 1)
  out_proj -> attn_out^T -> PE-transpose back + residual -> x1 (natural)
  LN2 -> h2^T ; MLP in 2 batch-groups (weights streamed once per group)
  quick-gelu == Gelu_apprx_sigmoid on ACT; fc2 out -> transpose + residual.
Matmuls run in float32r (full PE rate at N>=256; fp32 would be 4x slower).
Pools are stage-scoped (manually closed ExitStacks) to fit SBUF.
"""

from contextlib import ExitStack

import numpy as np

import concourse.bass as bass
import concourse.mybir as mybir
import concourse.tile as tile
from concourse import bacc, bass_utils
from concourse.masks import make_identity

N_CORES = 8
B, S, D = 32, 257, 1024
H, HD = 16, 64
FF = 4096
EPS = 1e-5
SCALE = HD ** -0.5
NB = B // N_CORES  # batch elems per core

F32 = mybir.dt.float32
F32R = mybir.dt.float32r
F16 = mybir.dt.float16
AF = mybir.ActivationFunctionType
ALU = mybir.AluOpType

# sequence chunks (partition-dim tiling of S=257)
SQ = [(0, 128), (128, 128), (256, 1)]
SE = 258  # free-dim padded length: fp32r matmul needs even moving/dst size
DC = D // 128   # 8 chunks of the model dim
FC = FF // 128  # 32 chunks of the ff dim


def r(ap):
    return ap.bitcast(F32R)


def build():
    nc = bacc.Bacc("TRN2", target_bir_lowering=False, debug=False,
                   num_devices=N_CORES)

    def din(name, shape, dt=F32):
        return nc.dram_tensor(name, shape, dt, kind="ExternalInput").ap()

    x_d = din("x", [NB, S, D])
    qwT_d = din("qwT", [D, D], F16)
    kwT_d = din("kwT", [D, D], F16)
    vwT_d = din("vwT", [D, D], F16)
    owT_d = din("owT", [D, D], F16)
    f1wT_d = din("f1wT", [D, FF], F16)
    f2wT_d = din("f2wT", [FF, D], F16)
    qb_d = din("qb", [D])
    kb_d = din("kb", [D])
    ob_d = din("ob", [D])
    f1b_d = din("f1b", [FF])
    f2b_d = din("f2b", [D])
    g1_d = din("g1", [D])
    b1_d = din("b1", [D])
    g2_d = din("g2", [D])
    b2_d = din("b2", [D])
    out_d = nc.dram_tensor("out", [NB, S, D], F32, kind="ExternalOutput").ap()

    with tile.TileContext(nc) as tc:
        with ExitStack() as es:
            P = lambda name, bufs, **kw: es.enter_context(
                tc.tile_pool(name=name, bufs=bufs, **kw))
            const = P("const", 1)
            biasp = P("bias", 1)
            xio = P("xio", 2)
            stat = P("stat", 8)
            pp = P("pp", 8, space="PSUM")
            pt = pp
            dramp = P("dram", 1, space="DRAM")

            ident = const.tile([128, 128], F32)
            make_identity(nc, ident)
            ident16 = const.tile([128, 128], F16)
            make_identity(nc, ident16)
            ones16 = const.tile([128, 1], F16)
            nc.vector.memset(ones16[:], 1.0)
            epsc = const.tile([128, 1], F32)
            nc.vector.memset(epsc[:], EPS)

            def load_bias(dram, n):
                t = biasp.tile([128, n // 128], F32, name=f"bias_{dram.name}")
                nc.sync.dma_start(t[:], dram.rearrange("(c p) -> p c", p=128))
                return t

            qb_sb = load_bias(qb_d, D)
            kb_sb = load_bias(kb_d, D)
            ob_sb = load_bias(ob_d, D)
            f1b_sb = load_bias(f1b_d, FF)
            f2b_sb = load_bias(f2b_d, D)
            g1_sb = load_bias(g1_d, D)
            b1_sb = load_bias(b1_d, D)
            g2_sb = load_bias(g2_d, D)
            b2_sb = load_bias(b2_d, D)

            x1_scr = dramp.tile([NB, S, D], F32)

            def layer_norm(src_tiles, hpool):
                """src_tiles: 3 natural tiles [(pz, D)]; returns normalized
                (x-mu)*rstd tiles (gamma/beta applied at transpose evict)."""
                out_tiles = []
                for j, (o, pz) in enumerate(SQ):
                    xt = src_tiles[j]
                    st = stat.tile([pz, 2, 6], F32, name="st", tag="st")
                    nc.vector.bn_stats(st[:, 0, :], xt[:, 0:512])
                    nc.vector.bn_stats(st[:, 1, :], xt[:, 512:1024])
                    mv = stat.tile([pz, 2], F32, name="mv", tag="mv")
                    nc.vector.bn_aggr(mv[:], st[:])
                    rstd = stat.tile([pz, 1], F32, name="rstd", tag="rstd")
                    nc.scalar.activation(rstd[:], mv[:, 1:2], AF.Sqrt,
                                         bias=epsc[:pz, :])
                    nc.vector.reciprocal(rstd[:], rstd[:])
                    ht = hpool.tile([pz, D], F32, name="hn", tag="hn")
                    nc.vector.tensor_scalar(
                        out=ht[:], in0=xt[:], scalar1=mv[:, 0:1],
                        scalar2=rstd[:], op0=ALU.subtract, op1=ALU.mult)
                    out_tiles.append(ht)
                return out_tiles

            def transpose_to_T(nat_tiles, dst_pool, g_sb, bt_sb, tag):
                """nat tiles [(pz, D)] -> 8 tiles [128, S] of the transpose,
                evicted with per-partition scale g and bias bt."""
                outs = []
                for dc in range(DC):
                    ps = pt.tile([128, SE], F32, name="psT", tag="pp")
                    for j, (o, pz) in enumerate(SQ):
                        nc.tensor.transpose(
                            ps[:, o:o + pz],
                            nat_tiles[j][:, dc * 128:(dc + 1) * 128],
                            ident[:pz, :pz])
                    t = dst_pool.tile([128, SE], F16, name=f"{tag}", tag=tag)
                    nc.vector.tensor_scalar(
                        out=t[:], in0=ps[:], scalar1=g_sb[:, dc:dc + 1],
                        scalar2=bt_sb[:, dc:dc + 1], op0=ALU.mult, op1=ALU.add)
                    outs.append(t)
                return outs

            def project_T(wT_dram, rhs_per_b, bias_sb, dst_pool, tag, wpool,
                          func=AF.Identity, odt=F16):
                """y^T = wT.T @ rhs (+bias) for every batch elem.
                Streams wT in two m-halves of [128, DC, 512] to bound SBUF.
                Returns outs[b][mc] tiles [128, S]."""
                src = wT_dram.rearrange("(kc p) m -> p kc m", p=128)
                outs = [[None] * DC for _ in range(NB)]
                for half in range(2):
                    wt = wpool.tile([128, DC, 512], F16, name="pw", tag="pw")
                    nc.sync.dma_start(
                        wt[:], src[:, :, half * 512:(half + 1) * 512])
                    for b in range(NB):
                        for ml in range(4):
                            mc = half * 4 + ml
                            ps = pp.tile([128, SE], F32, name="psP", tag="pp")
                            for kc in range(DC):
                                nc.tensor.matmul(
                                    ps[:],
                                    wt[:, kc, ml * 128:(ml + 1) * 128],
                                    rhs_per_b[b][kc][:],
                                    start=(kc == 0), stop=(kc == DC - 1))
                            t = dst_pool.tile([128, SE], odt, name=tag,
                                              tag=tag)
                            if func is AF.Identity:
                                nc.vector.tensor_scalar_add(
                                    t[:], ps[:], bias_sb[:, mc:mc + 1])
                            else:
                                nc.scalar.activation(t[:], ps[:], func,
                                                     bias=bias_sb[:, mc:mc + 1])
                            outs[b][mc] = t
                return outs

            # ---------- stage A: load x, LN1, h^T ----------
            esA_HT = ExitStack()
            HTp = esA_HT.enter_context(
                tc.tile_pool(name="HT", bufs=NB * DC, side="right"))
            esA = ExitStack()
            hnat = esA.enter_context(
                tc.tile_pool(name="hnat", bufs=3, side="right"))
            HT = []
            for b in range(NB):
                xts = []
                for j, (o, pz) in enumerate(SQ):
                    xt = xio.tile([pz, D], F32, name="xin", tag="xin")
                    nc.sync.dma_start(xt[:], x_d[b, o:o + pz, :])
                    xts.append(xt)
                hts = layer_norm(xts, hnat)
                HT.append(transpose_to_T(hts, HTp, g1_sb, b1_sb, "HT"))

            # ---------- stage B: QKV ----------
            esA.close()  # hnat dead
            esBC = ExitStack()
            qTp = esBC.enter_context(tc.tile_pool(name="qT", bufs=NB * DC))
            kTp = esBC.enter_context(tc.tile_pool(name="kT", bufs=NB * DC))
            vp = esBC.enter_context(tc.tile_pool(name="vna", bufs=NB * 3))
            esB = ExitStack()
            pwB = esB.enter_context(tc.tile_pool(name="pwB", bufs=3))

            qT = project_T(qwT_d, HT, qb_sb, qTp, "qT", pwB)
            kT = project_T(kwT_d, HT, kb_sb, kTp, "kT", pwB)

            # v in natural layout [s, D] (no bias: folded into attn eviction)
            vna = []
            v_src = vwT_d.rearrange("(kc p) m -> p kc m", p=128)
            for half in range(2):
                wt = pwB.tile([128, DC, 512], F16, name="pw", tag="pw")
                nc.sync.dma_start(
                    wt[:], v_src[:, :, half * 512:(half + 1) * 512])
                for b in range(NB):
                    if half == 0:
                        vna.append([vp.tile([pz, D], F16, name="vna",
                                            tag="vna") for (o, pz) in SQ])
                    for j, (o, pz) in enumerate(SQ):
                        ps = pp.tile([128, 512], F32, name="psV", tag="pp")
                        for kc in range(DC):
                            nc.tensor.matmul(
                                ps[:pz, :],
                                HT[b][kc][:, o:o + pz],
                                wt[:, kc, :],
                                start=(kc == 0), stop=(kc == DC - 1))
                        nc.vector.tensor_copy(
                            vna[b][j][:, half * 512:(half + 1) * 512],
                            ps[:pz, :])
            esB.close()    # qkv weights dead
            esA_HT.close()  # HT dead

            # ---------- stage C: attention ----------
            # scoresT = k_h @ q_h^T directly (no probs transpose); softmax
            # denominator via ones-matmul column sums; normalization fused
            # into the DVE eviction of attn^T; v_b folded into o_b on host.
            esC = ExitStack()
            probsTp = esC.enter_context(tc.tile_pool(name="probsT", bufs=8))
            rcp = esC.enter_context(tc.tile_pool(name="rcp", bufs=3))
            esCD = ExitStack()
            attnTp = esCD.enter_context(
                tc.tile_pool(name="attnT", bufs=NB * DC, side="right"))
            attnT = []
            for b in range(NB):
                attnT.append([None] * DC)
                # pass 1: scoresT, exp; per-head column sums land at psum
                # partitions 0/32/64/96 (PE tile_position) so one 128-lane
                # reciprocal serves 4 heads
                for h in range(H):
                    dc, po = h // 2, (h % 2) * 64
                    pTs = []
                    csum = pp.tile([1, SE], F32, name="psCS", tag="pp")
                    for sj, (so, spz) in enumerate(SQ):
                        scT = pp.tile([128, SE], F32, name="psS", tag="pp")
                        nc.tensor.matmul(
                            scT[:spz, :],
                            kT[b][dc][po:po + 64, so:so + spz],
                            qT[b][dc][po:po + 64, :],
                            start=True, stop=True)
                        pT = probsTp.tile([spz, SE], F16, name="pT", tag="pT")
                        nc.scalar.activation(pT[:], scT[:spz, :], AF.Exp)
                        pTs.append(pT)
                        nc.tensor.matmul(
                            csum[:], ones16[:spz, :], pTs[sj][:],
                            start=(sj == 0), stop=(sj == 2))
                    rc = rcp.tile([1, SE], F32, name="rc", tag="rc")
                    nc.vector.reciprocal_approx_fast(rc[:], csum[:])
                    rcb = rcp.tile([64, SE], F32, name="rcb", tag="rcb")
                    nc.gpsimd.partition_broadcast(rcb[:], rc[:])
                    at = pp.tile([64, SE], F32, name="psA", tag="pp")
                    for sj, (so, spz) in enumerate(SQ):
                        nc.tensor.matmul(
                            at[:], vna[b][sj][:, h * 64:(h + 1) * 64],
                            pTs[sj][:],
                            start=(sj == 0), stop=(sj == 2))
                    if po == 0:
                        attnT[b][dc] = attnTp.tile([128, SE], F16,
                                                   name="atT", tag="atT")
                    nc.vector.tensor_tensor(
                        out=attnT[b][dc][po:po + 64, :], in0=at[:],
                        in1=rcb[:], op=ALU.mult)
            esC.close()   # probsT, rcp dead
            esBC.close()  # qT, kT, vna dead

            # ---------- stage D: out_proj, residual, LN2, h2^T ----------
            esD2 = ExitStack()
            aoTp = esD2.enter_context(tc.tile_pool(name="aoT", bufs=NB * DC))
            esD = ExitStack()
            pwD = esD.enter_context(tc.tile_pool(name="pwD", bufs=3))
            aoT = project_T(owT_d, attnT, ob_sb, aoTp, "aoT", pwD, odt=F32)
            esD.close()   # ow weights dead
            esCD.close()  # attnT dead

            esDE = ExitStack()
            H2Tp = esDE.enter_context(
                tc.tile_pool(name="H2T", bufs=NB * DC, side="right"))
            esD3 = ExitStack()
            x1p = esD3.enter_context(
                tc.tile_pool(name="x1", bufs=3, side="right"))
            h2natp = esD3.enter_context(
                tc.tile_pool(name="h2nat", bufs=3, side="right"))
            H2T = []
            for b in range(NB):
                x1ts = []
                for j, (o, pz) in enumerate(SQ):
                    xres = xio.tile([pz, D], F32, name="xres", tag="xin")
                    nc.sync.dma_start(xres[:], x_d[b, o:o + pz, :])
                    x1t = x1p.tile([pz, D], F32, name="x1", tag="x1")
                    for hf in range(2):
                        ps = pt.tile([pz, 512], F32, name="psN", tag="pp")
                        for dl in range(4):
                            dc = hf * 4 + dl
                            nc.tensor.transpose(
                                ps[:, dl * 128:(dl + 1) * 128],
                                aoT[b][dc][:, o:o + pz], ident[:128, :128])
                        nc.vector.tensor_tensor(
                            out=x1t[:, hf * 512:(hf + 1) * 512], in0=ps[:],
                            in1=xres[:, hf * 512:(hf + 1) * 512], op=ALU.add)
                    nc.sync.dma_start(x1_scr[b, o:o + pz, :], x1t[:])
                    x1ts.append(x1t)
                h2ts = layer_norm(x1ts, h2natp)
                H2T.append(transpose_to_T(h2ts, H2Tp, g2_sb, b2_sb, "H2T"))
            esD2.close()  # aoT dead
            esD3.close()  # x1, h2nat dead

            # ---------- stage E: MLP in 2 batch groups ----------
            esE = ExitStack()
            w1p = esE.enter_context(tc.tile_pool(name="w1", bufs=2))
            w2p = esE.enter_context(tc.tile_pool(name="w2", bufs=3))
            h1Tp = esE.enter_context(tc.tile_pool(name="h1T", bufs=2 * FC))
            moTp = esE.enter_context(tc.tile_pool(name="moT", bufs=2 * DC))
            outnp = esE.enter_context(tc.tile_pool(name="outn", bufs=2))
            f1_src = f1wT_d.rearrange("(kc p) m -> p kc m", p=128)
            f2_src = f2wT_d.rearrange("(kc p) m -> p kc m", p=128)
            for grp in range(2):
                bs = [grp * 2, grp * 2 + 1]
                h1T = {b: [None] * FC for b in bs}
                for mc in range(FC):
                    w1t = w1p.tile([128, DC, 128], F16, name="w1", tag="w1")
                    nc.sync.dma_start(
                        w1t[:], f1_src[:, :, mc * 128:(mc + 1) * 128])
                    for b in bs:
                        ps = pp.tile([128, SE], F32, name="psF1", tag="pp")
                        for kc in range(DC):
                            nc.tensor.matmul(
                                ps[:], w1t[:, kc, :], H2T[b][kc][:],
                                start=(kc == 0), stop=(kc == DC - 1))
                        t = h1Tp.tile([128, SE], F16, name="h1T",
                                      tag="h1T")
                        nc.scalar.activation(t[:], ps[:],
                                             AF.Gelu_apprx_sigmoid,
                                             bias=f1b_sb[:, mc:mc + 1])
                        h1T[b][mc] = t
                moT = {b: [None] * DC for b in bs}
                for mc in range(DC):
                    for kh in range(2):
                        w2t = w2p.tile([128, FC // 2, 128], F16, name="w2",
                                       tag="w2")
                        nc.sync.dma_start(
                            w2t[:], f2_src[:, kh * 16:(kh + 1) * 16,
                                           mc * 128:(mc + 1) * 128])
                        if kh == 0:
                            ps2 = {b: pp.tile([128, SE], F32, name="psF2",
                                              tag="pp") for b in bs}
                        for b in bs:
                            for kc in range(FC // 2):
                                nc.tensor.matmul(
                                    ps2[b][:], w2t[:, kc, :],
                                    h1T[b][kh * 16 + kc][:],
                                    start=(kh == 0 and kc == 0),
                                    stop=(kh == 1 and kc == FC // 2 - 1))
                    for b in bs:
                        t = moTp.tile([128, SE], F32, name="moT", tag="moT")
                        nc.vector.tensor_scalar_add(t[:], ps2[b][:],
                                                    f2b_sb[:, mc:mc + 1])
                        moT[b][mc] = t
                for b in bs:
                    for j, (o, pz) in enumerate(SQ):
                        x1res = xio.tile([pz, D], F32, name="x1r", tag="xin")
                        nc.sync.dma_start(x1res[:], x1_scr[b, o:o + pz, :])
                        ot = outnp.tile([pz, D], F32, name="outn", tag="outn")
                        for hf in range(2):
                            ps = pt.tile([pz, 512], F32, name="psO", tag="pp")
                            for dl in range(4):
                                dc = hf * 4 + dl
                                nc.tensor.transpose(
                                    ps[:, dl * 128:(dl + 1) * 128],
                                    moT[b][dc][:, o:o + pz], ident[:128, :128])
                            nc.vector.tensor_tensor(
                                out=ot[:, hf * 512:(hf + 1) * 512], in0=ps[:],
                                in1=x1res[:, hf * 512:(hf + 1) * 512],
                                op=ALU.add)
                        nc.sync.dma_start(out_d[b, o:o + pz, :], ot[:])
            esE.close()
            esDE.close()

    nc.compile()
    return nc


_NC = None


def _get_nc():
    global _NC
    if _NC is None:
        _NC = build()
    return _NC


def _prep_inputs(inputs):
    f = lambda a: np.ascontiguousarray(np.asarray(a, dtype=np.float32))
    x = f(inputs["hidden_states"])
    h = lambda a: np.ascontiguousarray(a.astype(np.float16))
    shared = {
        "qwT": h(f(inputs["q_w"]).T * SCALE),
        "kwT": h(f(inputs["k_w"]).T),
        "vwT": h(f(inputs["v_w"]).T),
        "owT": h(f(inputs["o_w"]).T),
        "f1wT": h(f(inputs["fc1_w"]).T),
        "f2wT": h(f(inputs["fc2_w"]).T),
        "qb": f(inputs["q_b"]) * SCALE,
        "kb": f(inputs["k_b"]),
        "ob": f(inputs["o_b"]) + f(inputs["o_w"]) @ f(inputs["v_b"]),
        "f1b": f(inputs["fc1_b"]),
        "f2b": f(inputs["fc2_b"]),
        "g1": f(inputs["ln1_g"]),
        "b1": f(inputs["ln1_b"]),
        "g2": f(inputs["ln2_g"]),
        "b2": f(inputs["ln2_b"]),
    }
    shared = {k: np.ascontiguousarray(v) for k, v in shared.items()}
    in_maps = []
    for c in range(N_CORES):
        m = dict(shared)
        m["x"] = np.ascontiguousarray(x[c * NB:(c + 1) * NB])
        in_maps.append(m)
    return in_maps


def run(inputs, trace=False):
    nc = _get_nc()
    in_maps = _prep_inputs(inputs)
    res = bass_utils.run_bass_kernel_spmd(
        nc, in_maps, core_ids=list(range(N_CORES)), trace=trace)
    out = np.concatenate([res.results[c]["out"] for c in range(N_CORES)],
                         axis=0)
    return out, res


def kernel(**inputs):
    out, _ = run(inputs, trace=False)
    return out



# revision 4
# speedup vs baseline: 1.7782x; 1.7782x over previous
"""ChineseCLIPVisionLayer on 8 trn2 NeuronCores.

Sharding: pure data-parallel over batch (B=32 -> 4 per core), zero
collectives. Weights host-quantized to fp8e4 (x4096 power-of-2 scale,
undone at psum eviction) and host-packed into the exact SBUF tile
layout so every weight DMA is contiguous.

Per-core pipeline (activations feeding the big matmuls live in
transposed [D, S] fp8 tiles shaped [128, kc, 272] so DoubleRow matmuls
contract 256 rows per pass -> 2x PE throughput):
  LN1 -> h^T fp8 ; q^T,k^T f16 ; v natural f16 with a ones(=1/4) column
  per head appended so the attention matmul also yields the softmax
  denominator (no separate column-sum matmuls).
  attention: heads pipelined (attn MMs deferred one head) so PE never
  waits on ACT's exp; eviction scales by 4/csum -> attnT fp8 (x4).
  out_proj -> aoT f16 -> PE-transpose (f16, 1 cyc/row) + residual
  -> x1 (kept in SBUF) -> LN2 -> h2^T fp8 -> MLP single pass over all
  4 batch elems per weight tile; quick-gelu on ACT with x1/4096 scale;
  fc2 -> moT f16 -> transpose + residual -> out.
"""

from contextlib import ExitStack

import numpy as np

import concourse.bass as bass
import concourse.mybir as mybir
import concourse.tile as tile
from concourse import bacc, bass_utils
from concourse.masks import make_identity

N_CORES = 8
B, S, D = 32, 257, 1024
H, HD = 16, 64
FF = 4096
EPS = 1e-5
SCALE = HD ** -0.5
NB = B // N_CORES

F32 = mybir.dt.float32
F16 = mybir.dt.float16
F8 = mybir.dt.float8e4
AF = mybir.ActivationFunctionType
ALU = mybir.AluOpType
DR = mybir.MatmulPerfMode.DoubleRow

SQ = [(0, 128), (128, 128), (256, 1)]
SE = 258   # matmul moving/free length (S padded to even)
SP = 272   # fp8 activation tile stride (multiple of 16 for DoubleRow)
DC = D // 128
FC = FF // 128
KP = DC // 2          # k-pairs for a D-contraction
WS = 4096.0           # fp8 weight scale (power of 2)
QS = WS * 8.0         # q weight scale (x8 since q_w premultiplied by 1/8)
AS = 4.0              # attnT fp8 activation scale (via ones column 1/AS)


def build():
    nc = bacc.Bacc("TRN2", target_bir_lowering=False, debug=False,
                   num_devices=N_CORES)

    def din(name, shape, dt=F32):
        return nc.dram_tensor(name, shape, dt, kind="ExternalInput").ap()

    x_d = din("x", [NB, S, D])
    qw_d = din("qw8", [128, DC, D], F8)
    kw_d = din("kw8", [128, DC, D], F8)
    vw_d = din("vw8", [128, DC, D], F8)
    ow_d = din("ow8", [128, DC, D], F8)
    f1w_d = din("f1w8", [FC, 128, DC, 128], F8)
    f2w_d = din("f2w8", [DC, 128, FC, 128], F8)
    qb_d = din("qb", [D])
    kb_d = din("kb", [D])
    ob_d = din("ob", [D])
    f1b_d = din("f1b", [FF])
    f2b_d = din("f2b", [D])
    g1_d = din("g1", [D])
    b1_d = din("b1", [D])
    g2_d = din("g2", [D])
    b2_d = din("b2", [D])
    out_d = nc.dram_tensor("out", [NB, S, D], F32, kind="ExternalOutput").ap()

    with tile.TileContext(nc) as tc:
        with ExitStack() as es:
            P = lambda name, bufs, **kw: es.enter_context(
                tc.tile_pool(name=name, bufs=bufs, **kw))
            const = P("const", 1)
            biasp = P("bias", 1)
            xio = P("xio", 5)
            stat = P("stat", 8)
            pp = P("pp", 8, space="PSUM")

            ident16 = const.tile([128, 128], F16)
            make_identity(nc, ident16)
            epsc = const.tile([128, 1], F32)
            nc.vector.memset(epsc[:], EPS)

            def load_bias(dram, n):
                t = biasp.tile([128, n // 128], F32, name=f"bias_{dram.name}")
                nc.sync.dma_start(t[:], dram.rearrange("(c p) -> p c", p=128))
                return t

            qb_sb = load_bias(qb_d, D)
            kb_sb = load_bias(kb_d, D)
            ob_sb = load_bias(ob_d, D)
            f1b_sb = load_bias(f1b_d, FF)
            f2b_sb = load_bias(f2b_d, D)
            g1_sb = load_bias(g1_d, D)
            b1_sb = load_bias(b1_d, D)
            g2_sb = load_bias(g2_d, D)
            b2_sb = load_bias(b2_d, D)

            def layer_norm(src_tiles, hpool):
                """src: 3 natural f32 tiles [(pz, D)]; returns f16 tiles of
                (x-mu)*rstd (gamma/beta applied at transpose evict)."""
                out_tiles = []
                for j, (o, pz) in enumerate(SQ):
                    xt = src_tiles[j]
                    st = stat.tile([pz, 2, 6], F32, name="st", tag="st")
                    nc.vector.bn_stats(st[:, 0, :], xt[:, 0:512])
                    nc.vector.bn_stats(st[:, 1, :], xt[:, 512:1024])
                    mv = stat.tile([pz, 2], F32, name="mv", tag="mv")
                    nc.vector.bn_aggr(mv[:], st[:])
                    rstd = stat.tile([pz, 1], F32, name="rstd", tag="rstd")
                    nc.scalar.activation(rstd[:], mv[:, 1:2], AF.Sqrt,
                                         bias=epsc[:pz, :])
                    nc.vector.reciprocal(rstd[:], rstd[:])
                    ht = hpool.tile([pz, D], F16, name="hn", tag="hn")
                    nc.vector.tensor_scalar(
                        out=ht[:], in0=xt[:], scalar1=mv[:, 0:1],
                        scalar2=rstd[:], op0=ALU.subtract, op1=ALU.mult)
                    out_tiles.append(ht)
                return out_tiles

            def transpose_to_T8(nat_tiles, dst8, g_sb, bt_sb):
                """nat f16 tiles [(pz, D)] -> dst8 [128, DC, SP] fp8 holding
                the transpose, scaled by per-partition g and biased bt."""
                for dc in range(DC):
                    ps = pp.tile([128, SE], F16, name="psT", tag="pp")
                    for j, (o, pz) in enumerate(SQ):
                        nc.tensor.transpose(
                            ps[:, o:o + pz],
                            nat_tiles[j][:, dc * 128:(dc + 1) * 128],
                            ident16[:pz, :pz])
                    nc.vector.tensor_scalar(
                        out=dst8[:, dc, 0:SE], in0=ps[:],
                        scalar1=g_sb[:, dc:dc + 1], scalar2=bt_sb[:, dc:dc + 1],
                        op0=ALU.mult, op1=ALU.add)

            def project_dr(wt, rhs8, bias_sb, inv_s, dst_pool, tag,
                           func=None, odt=F16, dst8=None):
                """y^T[mc] = (wT.T @ rhs)/inv_s + bias for one batch elem.
                wt: [128, DC, D] fp8 weight tile; rhs8: [128, DC, SP] fp8.
                Returns 8 tiles [128, SE] (or writes dst8 [128, DC, SP])."""
                outs = []
                for mc in range(DC):
                    ps = pp.tile([128, SE], F32, name="psP", tag="pp")
                    for kp in range(KP):
                        nc.tensor.matmul(
                            ps[:],
                            wt[:, 2 * kp:2 * kp + 2, mc * 128:(mc + 1) * 128],
                            rhs8[:, 2 * kp:2 * kp + 2, 0:SE],
                            start=(kp == 0), stop=(kp == KP - 1),
                            perf_mode=DR)
                    if dst8 is not None:
                        nc.vector.tensor_scalar(
                            out=dst8[:, mc, 0:SE], in0=ps[:], scalar1=inv_s,
                            scalar2=bias_sb[:, mc:mc + 1],
                            op0=ALU.mult, op1=ALU.add)
                    elif func is not None:
                        t = dst_pool.tile([128, SE], odt, name=tag, tag=tag)
                        nc.scalar.activation(t[:], ps[:], func,
                                             bias=bias_sb[:, mc:mc + 1],
                                             scale=inv_s)
                        outs.append(t)
                    else:
                        t = dst_pool.tile([128, SE], odt, name=tag, tag=tag)
                        nc.vector.tensor_scalar(
                            out=t[:], in0=ps[:], scalar1=inv_s,
                            scalar2=bias_sb[:, mc:mc + 1],
                            op0=ALU.mult, op1=ALU.add)
                        outs.append(t)
                return outs

            # ---------- stage A: load x, LN1, h^T fp8 ----------
            esA_HT = ExitStack()
            HT8p = esA_HT.enter_context(
                tc.tile_pool(name="HT8", bufs=NB, side="right"))
            esA = ExitStack()
            hnat = esA.enter_context(
                tc.tile_pool(name="hnat", bufs=4, side="right"))
            HT8 = []
            for b in range(NB):
                xts = []
                for j, (o, pz) in enumerate(SQ):
                    xt = xio.tile([pz, D], F32, name="xin", tag="xin")
                    nc.sync.dma_start(xt[:], x_d[b, o:o + pz, :])
                    xts.append(xt)
                hts = layer_norm(xts, hnat)
                t8 = HT8p.tile([128, DC, SP], F8, name="HT8", tag="HT8")
                transpose_to_T8(hts, t8, g1_sb, b1_sb)
                HT8.append(t8)

            # ---------- stage B: QKV ----------
            esA.close()
            esBC = ExitStack()
            qTp = esBC.enter_context(tc.tile_pool(name="qT", bufs=NB * DC))
            kTp = esBC.enter_context(tc.tile_pool(name="kT", bufs=NB * DC))
            vp = esBC.enter_context(tc.tile_pool(name="vna", bufs=NB * 3))
            esB = ExitStack()
            pwB = esB.enter_context(tc.tile_pool(name="pwB", bufs=2))

            def load_w(dram):
                wt = pwB.tile([128, DC, D], F8, name="pw", tag="pw")
                nc.sync.dma_start(wt[:], dram[:, :, :])
                return wt

            qwt = load_w(qw_d)
            qT = [project_dr(qwt, HT8[b], qb_sb, 1.0 / QS, qTp, "qT")
                  for b in range(NB)]
            kwt = load_w(kw_d)
            kT = [project_dr(kwt, HT8[b], kb_sb, 1.0 / WS, kTp, "kT")
                  for b in range(NB)]

            # v natural [s, 16, 65] f16; col 64 of each head = 1/AS so the
            # attention matmul's row 64 is csum/AS (no bias: folded into o_b)
            vwt = load_w(vw_d)
            vna = []
            for b in range(NB):
                vb = []
                for j, (o, pz) in enumerate(SQ):
                    vt = vp.tile([pz, H, HD + 1], F16, name="vna", tag="vna")
                    nc.vector.memset(vt[:, :, HD:HD + 1], 1.0 / AS)
                    for half in range(2):
                        ps = pp.tile([pz, 512], F32, name="psV", tag="pp")
                        for kp in range(KP):
                            nc.tensor.matmul(
                                ps[:, :],
                                HT8[b][:, 2 * kp:2 * kp + 2, o:o + pz],
                                vwt[:, 2 * kp:2 * kp + 2,
                                    half * 512:(half + 1) * 512],
                                start=(kp == 0), stop=(kp == KP - 1),
                                perf_mode=DR)
                        nc.vector.tensor_scalar(
                            out=vt[:, half * 8:(half + 1) * 8, 0:HD],
                            in0=ps[:, :], scalar1=1.0 / WS, scalar2=None,
                            op0=ALU.mult)
                    vb.append(vt)
                vna.append(vb)
            esB.close()
            esA_HT.close()  # HT8 dead

            # ---------- stage C+D: attention (pipelined heads), out_proj,
            # residual, LN2, h2^T; interleaved per batch elem ----------
            esCD = ExitStack()
            probsTp = esCD.enter_context(tc.tile_pool(name="probsT", bufs=9))
            rcp = esCD.enter_context(tc.tile_pool(name="rcp", bufs=3))
            rcbp = esCD.enter_context(tc.tile_pool(name="rcb", bufs=3))
            atT8p = esCD.enter_context(tc.tile_pool(name="atT8", bufs=2))
            aoTp = esCD.enter_context(tc.tile_pool(name="aoT", bufs=2 * DC))
            pwD = esCD.enter_context(tc.tile_pool(name="pwD", bufs=1))
            esDE = ExitStack()
            H2T8p = esDE.enter_context(
                tc.tile_pool(name="H2T8", bufs=NB, side="right"))
            x1p = esDE.enter_context(
                tc.tile_pool(name="x1", bufs=NB * 3, side="right"))
            esD3 = ExitStack()
            # bufs=7: h2nat(b) is consumed by emit_h2t(b), deferred one b
            # later, so two b's worth of LN tiles must stay alive
            h2natp = esD3.enter_context(
                tc.tile_pool(name="h2nat", bufs=7, side="right"))

            owt = pwD.tile([128, DC, D], F8, name="pwO", tag="pwO")
            nc.sync.dma_start(owt[:], ow_d[:, :, :])

            attnT8 = [None] * NB
            aoT = [None] * NB
            x1 = [[None] * 3 for _ in range(NB)]
            h2nat = [None] * NB
            H2T8 = [None] * NB

            def emit_attention(b):
                attnT8[b] = atT8p.tile([128, DC, SP], F8, name="atT8",
                                       tag="atT8")
                pend = [None] * H  # per-head [3 pT tiles]

                def emit_attn_mm(h):
                    dc, po = h // 2, (h % 2) * 64
                    pTs = pend[h]
                    at = pp.tile([HD + 1, SE], F32, name="psA", tag="pp")
                    for sj, (so, spz) in enumerate(SQ):
                        nc.tensor.matmul(
                            at[:], vna[b][sj][:, h, :], pTs[sj][:],
                            start=(sj == 0), stop=(sj == 2))
                    # csum row must move to SBUF before reciprocal: the
                    # custom-DVE op mishandles psum base-partition offsets
                    cs = rcp.tile([1, SE], F32, name="cs", tag="cs")
                    nc.vector.tensor_copy(cs[:], at[HD:HD + 1, :])
                    rc = rcp.tile([1, SE], F32, name="rc", tag="rc")
                    nc.vector.reciprocal_approx_fast(rc[:], cs[:])
                    rcb = rcbp.tile([HD, SE], F32, name="rcb", tag="rcb")
                    nc.gpsimd.partition_broadcast(rcb[:], rc[:])
                    nc.vector.tensor_tensor(
                        out=attnT8[b][po:po + HD, dc, 0:SE],
                        in0=at[0:HD, :], in1=rcb[:], op=ALU.mult)

                for h in range(H):
                    dc, po = h // 2, (h % 2) * 64
                    pTs = []
                    for sj, (so, spz) in enumerate(SQ):
                        scT = pp.tile([spz, SE], F32, name="psS", tag="pp")
                        nc.tensor.matmul(
                            scT[:],
                            kT[b][dc][po:po + 64, so:so + spz],
                            qT[b][dc][po:po + 64, :],
                            start=True, stop=True)
                        pT = probsTp.tile([spz, SE], F16, name="pT", tag="pT")
                        nc.scalar.activation(pT[:], scT[:], AF.Exp)
                        pTs.append(pT)
                    pend[h] = pTs
                    if h > 0:
                        emit_attn_mm(h - 1)
                emit_attn_mm(H - 1)

            def emit_outproj(b):
                aoT[b] = project_dr(owt, attnT8[b], ob_sb, 1.0 / WS,
                                    aoTp, "aoT")

            def emit_natT_ln2(b):
                x1ts = []
                for j, (o, pz) in enumerate(SQ):
                    xres = xio.tile([pz, D], F32, name="xres", tag="xin")
                    nc.sync.dma_start(xres[:], x_d[b, o:o + pz, :])
                    x1t = x1p.tile([pz, D], F32, name="x1", tag="x1")
                    for hf in range(2):
                        ps = pp.tile([pz, 512], F16, name="psN", tag="pp")
                        for dl in range(4):
                            dc = hf * 4 + dl
                            nc.tensor.transpose(
                                ps[:, dl * 128:(dl + 1) * 128],
                                aoT[b][dc][:, o:o + pz], ident16[:128, :128])
                        nc.vector.tensor_tensor(
                            out=x1t[:, hf * 512:(hf + 1) * 512], in0=ps[:],
                            in1=xres[:, hf * 512:(hf + 1) * 512], op=ALU.add)
                    x1ts.append(x1t)
                x1[b] = x1ts
                h2nat[b] = layer_norm(x1ts, h2natp)

            def emit_h2t(b):
                t8 = H2T8p.tile([128, DC, SP], F8, name="H2T8", tag="H2T8")
                transpose_to_T8(h2nat[b], t8, g2_sb, b2_sb)
                H2T8[b] = t8

            for b in range(NB):
                emit_attention(b)
                emit_outproj(b)
                emit_natT_ln2(b)
                if b > 0:
                    emit_h2t(b - 1)
            emit_h2t(NB - 1)
            esCD.close()
            esBC.close()
            esD3.close()

            # ---------- stage E: MLP, single weight pass over all b ----------
            esE = ExitStack()
            w1p = esE.enter_context(tc.tile_pool(name="w1", bufs=3))
            w2p = esE.enter_context(tc.tile_pool(name="w2", bufs=2))
            h1Tp = esE.enter_context(tc.tile_pool(name="h1T", bufs=NB))
            moTp = esE.enter_context(tc.tile_pool(name="moT", bufs=NB * DC))
            outnp = esE.enter_context(tc.tile_pool(name="outn", bufs=3))

            h1T8 = [h1Tp.tile([128, FC, SP], F8, name="h1T8", tag="h1T8")
                    for b in range(NB)]
            for mc in range(FC):
                w1t = w1p.tile([128, DC, 128], F8, name="w1", tag="w1")
                nc.sync.dma_start(w1t[:], f1w_d[mc])
                for b in range(NB):
                    ps = pp.tile([128, SE], F32, name="psF1", tag="pp")
                    for kp in range(KP):
                        nc.tensor.matmul(
                            ps[:], w1t[:, 2 * kp:2 * kp + 2, :],
                            H2T8[b][:, 2 * kp:2 * kp + 2, 0:SE],
                            start=(kp == 0), stop=(kp == KP - 1),
                            perf_mode=DR)
                    nc.scalar.activation(h1T8[b][:, mc, 0:SE], ps[:],
                                         AF.Gelu_apprx_sigmoid,
                                         bias=f1b_sb[:, mc:mc + 1],
                                         scale=1.0 / WS)
            moT = [[None] * DC for _ in range(NB)]
            for mc in range(DC):
                w2t = w2p.tile([128, FC, 128], F8, name="w2", tag="w2")
                nc.sync.dma_start(w2t[:], f2w_d[mc])
                for b in range(NB):
                    ps = pp.tile([128, SE], F32, name="psF2", tag="pp")
                    for kp in range(FC // 2):
                        nc.tensor.matmul(
                            ps[:], w2t[:, 2 * kp:2 * kp + 2, :],
                            h1T8[b][:, 2 * kp:2 * kp + 2, 0:SE],
                            start=(kp == 0), stop=(kp == FC // 2 - 1),
                            perf_mode=DR)
                    t = moTp.tile([128, SE], F16, name="moT", tag="moT")
                    nc.vector.tensor_scalar(
                        out=t[:], in0=ps[:], scalar1=1.0 / WS,
                        scalar2=f2b_sb[:, mc:mc + 1],
                        op0=ALU.mult, op1=ALU.add)
                    moT[b][mc] = t
            for b in range(NB):
                for j, (o, pz) in enumerate(SQ):
                    ot = outnp.tile([pz, D], F32, name="outn", tag="outn")
                    for hf in range(2):
                        ps = pp.tile([pz, 512], F16, name="psO", tag="pp")
                        for dl in range(4):
                            dc = hf * 4 + dl
                            nc.tensor.transpose(
                                ps[:, dl * 128:(dl + 1) * 128],
                                moT[b][dc][:, o:o + pz], ident16[:128, :128])
                        nc.vector.tensor_tensor(
                            out=ot[:, hf * 512:(hf + 1) * 512], in0=ps[:],
                            in1=x1[b][j][:, hf * 512:(hf + 1) * 512],
                            op=ALU.add)
                    nc.sync.dma_start(out_d[b, o:o + pz, :], ot[:])
            esE.close()
            esDE.close()

    nc.compile()
    return nc


_NC = None


def _get_nc():
    global _NC
    if _NC is None:
        _NC = build()
    return _NC


def _q8(w, scale):
    """Quantize to TRN fp8e4 (e4m3, +-240) with a power-of-2 scale."""
    import ml_dtypes
    q = np.clip(w * scale, -240.0, 240.0).astype(ml_dtypes.float8_e4m3fn)
    return q.view(np.uint8)


def _pack_dd(w8):
    """[D, M] (wT layout, quantized) -> [128, DC, M] p-major tile."""
    return np.ascontiguousarray(
        w8.reshape(DC, 128, w8.shape[1]).transpose(1, 0, 2))


def _prep_inputs(inputs):
    f = lambda a: np.ascontiguousarray(np.asarray(a, dtype=np.float32))
    x = f(inputs["hidden_states"])
    qw8 = _q8(f(inputs["q_w"]).T * SCALE, QS)
    kw8 = _q8(f(inputs["k_w"]).T, WS)
    vw8 = _q8(f(inputs["v_w"]).T, WS)
    ow8 = _q8(f(inputs["o_w"]).T / AS, WS)
    f1w8 = _q8(f(inputs["fc1_w"]).T, WS)   # [D, FF]
    f2w8 = _q8(f(inputs["fc2_w"]).T, WS)   # [FF, D]
    shared = {
        "qw8": _pack_dd(qw8),
        "kw8": _pack_dd(kw8),
        "vw8": _pack_dd(vw8),
        "ow8": _pack_dd(ow8),
        # [D, FF] -> [FC, 128(p), DC(kc), 128(ml)]
        "f1w8": np.ascontiguousarray(
            f1w8.reshape(DC, 128, FC, 128).transpose(2, 1, 0, 3)),
        # [FF, D] -> [DC, 128(p), FC(kc), 128(ml)]
        "f2w8": np.ascontiguousarray(
            f2w8.reshape(FC, 128, DC, 128).transpose(2, 1, 0, 3)),
        "qb": f(inputs["q_b"]) * SCALE,
        "kb": f(inputs["k_b"]),
        "ob": f(inputs["o_b"]) + f(inputs["o_w"]) @ f(inputs["v_b"]),
        "f1b": f(inputs["fc1_b"]),
        "f2b": f(inputs["fc2_b"]),
        "g1": f(inputs["ln1_g"]),
        "b1": f(inputs["ln1_b"]),
        "g2": f(inputs["ln2_g"]),
        "b2": f(inputs["ln2_b"]),
    }
    shared = {k: np.ascontiguousarray(v) for k, v in shared.items()}
    in_maps = []
    for c in range(N_CORES):
        m = dict(shared)
        m["x"] = np.ascontiguousarray(x[c * NB:(c + 1) * NB])
        in_maps.append(m)
    return in_maps


def run(inputs, trace=False):
    nc = _get_nc()
    in_maps = _prep_inputs(inputs)
    res = bass_utils.run_bass_kernel_spmd(
        nc, in_maps, core_ids=list(range(N_CORES)), trace=trace)
    out = np.concatenate([res.results[c]["out"] for c in range(N_CORES)],
                         axis=0)
    return out, res


def kernel(**inputs):
    out, _ = run(inputs, trace=False)
    return out


# revision 30
# speedup vs baseline: 1.7835x; 1.0030x over previous
"""ChineseCLIPVisionLayer on 8 trn2 NeuronCores.

Sharding: pure data-parallel over batch (B=32 -> 4 per core), zero
collectives. Weights host-quantized to fp8e4 (x4096 power-of-2 scale,
undone at psum eviction) and host-packed into the exact SBUF tile
layout so every weight DMA is contiguous.

Per-core pipeline (activations feeding the big matmuls live in
transposed [D, S] fp8 tiles shaped [128, kc, 272] so DoubleRow matmuls
contract 256 rows per pass -> 2x PE throughput):
  LN1 -> h^T fp8 ; q^T,k^T f16 ; v natural f16 with a ones(=1/4) column
  per head appended so the attention matmul also yields the softmax
  denominator (no separate column-sum matmuls).
  attention: heads pipelined (attn MMs deferred one head) so PE never
  waits on ACT's exp; eviction scales by 4/csum -> attnT fp8 (x4).
  out_proj -> aoT f16 -> PE-transpose (f16, 1 cyc/row) + residual
  -> x1 (kept in SBUF) -> LN2 -> h2^T fp8 -> MLP single pass over all
  4 batch elems per weight tile; quick-gelu on ACT with x1/4096 scale;
  fc2 -> moT f16 -> transpose + residual -> out.
"""

from contextlib import ExitStack

import numpy as np

import concourse.bass as bass
import concourse.mybir as mybir
import concourse.tile as tile
from concourse import bacc, bass_utils
from concourse.masks import make_identity

N_CORES = 8
B, S, D = 32, 257, 1024
H, HD = 16, 64
FF = 4096
EPS = 1e-5
SCALE = HD ** -0.5
NB = B // N_CORES

F32 = mybir.dt.float32
F16 = mybir.dt.float16
F8 = mybir.dt.float8e4
I32 = mybir.dt.int32
AF = mybir.ActivationFunctionType
ALU = mybir.AluOpType
DR = mybir.MatmulPerfMode.DoubleRow

SQ = [(0, 128), (128, 128), (256, 1)]
SE = 258   # matmul moving/free length (S padded to even)
SP = 272   # fp8 activation tile stride (multiple of 16 for DoubleRow)
DC = D // 128
FC = FF // 128
KP = DC // 2          # k-pairs for a D-contraction
WS = 4096.0           # fp8 weight scale (power of 2)
QS = WS * 8.0         # q weight scale (x8 since q_w premultiplied by 1/8)
AS = 4.0              # attnT fp8 activation scale (via ones column 1/AS)
INTERLEAVE = True     # interleave out_proj(b-1) into attention(b) emission
J2PACK = True         # pack last-key scores/exp for head pairs at part 0/32
FUSE_EXP = True       # one exp over a [128,2,512] double-bank psum tile


def build():
    nc = bacc.Bacc("TRN2", target_bir_lowering=False, debug=False,
                   num_devices=N_CORES)

    def din(name, shape, dt=F32):
        return nc.dram_tensor(name, shape, dt, kind="ExternalInput").ap()

    x_d = din("x", [NB, S, D])
    qw_d = din("qw8", [128, DC, D], F8)
    kw_d = din("kw8", [128, DC, D], F8)
    vw_d = din("vw8", [128, DC, D], F8)
    ow_d = din("ow8", [128, DC, D], F8)
    f1w_d = din("f1w8", [FC, 128, DC, 128], F8)
    f2w_d = din("f2w8", [DC, 128, FC, 128], F8)
    qb_d = din("qb", [D])
    kb_d = din("kb", [D])
    ob_d = din("ob", [D])
    f1b_d = din("f1b", [FF])
    f2b_d = din("f2b", [D])
    g1_d = din("g1", [D])
    b1_d = din("b1", [D])
    g2_d = din("g2", [D])
    b2_d = din("b2", [D])
    out_d = nc.dram_tensor("out", [NB, S, D], F32, kind="ExternalOutput").ap()

    with tile.TileContext(nc) as tc:
        with ExitStack() as es:
            P = lambda name, bufs, **kw: es.enter_context(
                tc.tile_pool(name=name, bufs=bufs, **kw))
            const = P("const", 1)
            biasp = P("bias", 1)
            xio = P("xio", 5)
            stat = P("stat", 8)
            if FUSE_EXP:
                pp = P("pp", 4, space="PSUM")
                pq = P("pq", 2, space="PSUM")  # double-bank tiles
            else:
                pp = P("pp", 8, space="PSUM")

            ident16 = const.tile([128, 128], F16)
            make_identity(nc, ident16)
            epsc = const.tile([128, 1], F32)
            nc.vector.memset(epsc[:], EPS)

            def load_bias(dram, n):
                t = biasp.tile([128, n // 128], F32, name=f"bias_{dram.name}")
                nc.sync.dma_start(t[:], dram.rearrange("(c p) -> p c", p=128))
                return t

            qb_sb = load_bias(qb_d, D)
            kb_sb = load_bias(kb_d, D)
            ob_sb = load_bias(ob_d, D)
            f1b_sb = load_bias(f1b_d, FF)
            f2b_sb = load_bias(f2b_d, D)
            g1_sb = load_bias(g1_d, D)
            b1_sb = load_bias(b1_d, D)
            g2_sb = load_bias(g2_d, D)
            b2_sb = load_bias(b2_d, D)

            def layer_norm(src_tiles, hpool):
                """src: 3 natural f32 tiles [(pz, D)]; returns f16 tiles of
                (x-mu)*rstd (gamma/beta applied at transpose evict)."""
                out_tiles = []
                for j, (o, pz) in enumerate(SQ):
                    xt = src_tiles[j]
                    st = stat.tile([pz, 2, 6], F32, name="st", tag="st")
                    nc.vector.bn_stats(st[:, 0, :], xt[:, 0:512])
                    nc.vector.bn_stats(st[:, 1, :], xt[:, 512:1024])
                    mv = stat.tile([pz, 2], F32, name="mv", tag="mv")
                    nc.vector.bn_aggr(mv[:], st[:])
                    # rsqrt(var+eps) on DVE (Quake seed + 1 Newton) so ACT
                    # never loads the Sqrt table set between Exp phases
                    rstd = stat.tile([pz, 1], F32, name="rstd", tag="rstd")
                    ve = stat.tile([pz, 1], F32, name="ve", tag="ve")
                    nc.vector.tensor_scalar(
                        out=ve[:], in0=mv[:, 1:2], scalar1=EPS, scalar2=None,
                        op0=ALU.add)
                    nc.vector.tensor_scalar(
                        out=rstd[:].bitcast(I32), in0=ve[:].bitcast(I32),
                        scalar1=1, scalar2=-1,
                        op0=ALU.logical_shift_right, op1=ALU.bitwise_xor)
                    nc.vector.tensor_scalar(
                        out=rstd[:].bitcast(I32), in0=rstd[:].bitcast(I32),
                        scalar1=0x5F3759E0, scalar2=None, op0=ALU.add)
                    nr = stat.tile([pz, 1], F32, name="nr", tag="nr")
                    nc.vector.tensor_tensor(out=nr[:], in0=rstd[:],
                                            in1=rstd[:], op=ALU.mult)
                    nc.vector.tensor_tensor(out=nr[:], in0=nr[:], in1=ve[:],
                                            op=ALU.mult)
                    nc.vector.tensor_scalar(
                        out=nr[:], in0=nr[:], scalar1=-0.5, scalar2=1.5,
                        op0=ALU.mult, op1=ALU.add)
                    nc.vector.tensor_tensor(out=rstd[:], in0=rstd[:],
                                            in1=nr[:], op=ALU.mult)
                    ht = hpool.tile([pz, D], F16, name="hn", tag="hn")
                    nc.vector.tensor_scalar(
                        out=ht[:], in0=xt[:], scalar1=mv[:, 0:1],
                        scalar2=rstd[:], op0=ALU.subtract, op1=ALU.mult)
                    out_tiles.append(ht)
                return out_tiles

            def transpose_to_T8(nat_tiles, dst8, g_sb, bt_sb):
                """nat f16 tiles [(pz, D)] -> dst8 [128, DC, SP] fp8 holding
                the transpose, scaled by per-partition g and biased bt."""
                for dc in range(DC):
                    ps = pp.tile([128, SE], F16, name="psT", tag="pp")
                    for j, (o, pz) in enumerate(SQ):
                        nc.tensor.transpose(
                            ps[:, o:o + pz],
                            nat_tiles[j][:, dc * 128:(dc + 1) * 128],
                            ident16[:pz, :pz])
                    nc.vector.tensor_scalar(
                        out=dst8[:, dc, 0:SE], in0=ps[:],
                        scalar1=g_sb[:, dc:dc + 1], scalar2=bt_sb[:, dc:dc + 1],
                        op0=ALU.mult, op1=ALU.add)

            def project_dr(wt, rhs8, bias_sb, inv_s, dst_pool, tag,
                           func=None, odt=F16, dst8=None, mcs=None):
                """y^T[mc] = (wT.T @ rhs)*inv_s + bias for one batch elem.
                wt: [128, DC, D] fp8 weight tile; rhs8: [128, DC, SP] fp8.
                Returns tiles [128, SE] (or writes dst8 [128, DC, SP])."""
                outs = []
                for mc in (range(DC) if mcs is None else mcs):
                    ps = pp.tile([128, SE], F32, name="psP", tag="pp")
                    for kp in range(KP):
                        nc.tensor.matmul(
                            ps[:],
                            wt[:, 2 * kp:2 * kp + 2, mc * 128:(mc + 1) * 128],
                            rhs8[:, 2 * kp:2 * kp + 2, 0:SE],
                            start=(kp == 0), stop=(kp == KP - 1),
                            perf_mode=DR)
                    if dst8 is not None:
                        nc.vector.tensor_scalar(
                            out=dst8[:, mc, 0:SE], in0=ps[:], scalar1=inv_s,
                            scalar2=bias_sb[:, mc:mc + 1],
                            op0=ALU.mult, op1=ALU.add)
                    elif func is not None:
                        t = dst_pool.tile([128, SE], odt, name=tag, tag=tag)
                        nc.scalar.activation(t[:], ps[:], func,
                                             bias=bias_sb[:, mc:mc + 1],
                                             scale=inv_s)
                        outs.append(t)
                    else:
                        t = dst_pool.tile([128, SE], odt, name=tag, tag=tag)
                        nc.vector.tensor_scalar(
                            out=t[:], in0=ps[:], scalar1=inv_s,
                            scalar2=bias_sb[:, mc:mc + 1],
                            op0=ALU.mult, op1=ALU.add)
                        outs.append(t)
                return outs

            # ---------- stage A: load x, LN1, h^T fp8 ----------
            esA_HT = ExitStack()
            HT8p = esA_HT.enter_context(
                tc.tile_pool(name="HT8", bufs=NB, side="right"))
            esA = ExitStack()
            hnat = esA.enter_context(
                tc.tile_pool(name="hnat", bufs=4, side="right"))
            HT8 = []
            for b in range(NB):
                xts = []
                for j, (o, pz) in enumerate(SQ):
                    xt = xio.tile([pz, D], F32, name="xin", tag="xin")
                    nc.sync.dma_start(xt[:], x_d[b, o:o + pz, :])
                    xts.append(xt)
                hts = layer_norm(xts, hnat)
                t8 = HT8p.tile([128, DC, SP], F8, name="HT8", tag="HT8")
                transpose_to_T8(hts, t8, g1_sb, b1_sb)
                HT8.append(t8)

            # ---------- stage B: QKV ----------
            esA.close()
            esBC = ExitStack()
            qTp = esBC.enter_context(tc.tile_pool(name="qT", bufs=NB * DC))
            kTp = esBC.enter_context(tc.tile_pool(name="kT", bufs=NB * DC))
            vp = esBC.enter_context(tc.tile_pool(name="vna", bufs=NB * 3))
            esB = ExitStack()
            pwB = esB.enter_context(tc.tile_pool(name="pwB", bufs=2))

            def load_w(dram):
                wt = pwB.tile([128, DC, D], F8, name="pw", tag="pw")
                nc.sync.dma_start(wt[:], dram[:, :, :])
                return wt

            qwt = load_w(qw_d)
            qT = [project_dr(qwt, HT8[b], qb_sb, 1.0 / QS, qTp, "qT")
                  for b in range(NB)]
            kwt = load_w(kw_d)
            kT = [project_dr(kwt, HT8[b], kb_sb, 1.0 / WS, kTp, "kT")
                  for b in range(NB)]

            # v natural [s, 16, 65] f16; col 64 of each head = 1/AS so the
            # attention matmul's row 64 is csum/AS (no bias: folded into o_b).
            # The single s=256 row (j2) is duplicated at partitions 0 and 32
            # so head pairs can share one packed [33, SE] scores-psum + exp.
            vwt = load_w(vw_d)
            vna = []
            for b in range(NB):
                vb = []
                for j, (o, pz) in enumerate(SQ):
                    pv = 33 if (pz == 1 and J2PACK) else pz
                    vt = vp.tile([pv, H, HD + 1], F16, name="vna", tag="vna")
                    nc.vector.memset(vt[0:pz, :, HD:HD + 1], 1.0 / AS)
                    if pv == 33:
                        nc.vector.memset(vt[32:33, :, HD:HD + 1], 1.0 / AS)
                    for half in range(2):
                        ps = pp.tile([pz, 512], F32, name="psV", tag="pp")
                        for kp in range(KP):
                            nc.tensor.matmul(
                                ps[:, :],
                                HT8[b][:, 2 * kp:2 * kp + 2, o:o + pz],
                                vwt[:, 2 * kp:2 * kp + 2,
                                    half * 512:(half + 1) * 512],
                                start=(kp == 0), stop=(kp == KP - 1),
                                perf_mode=DR)
                        nc.vector.tensor_scalar(
                            out=vt[0:pz, half * 8:(half + 1) * 8, 0:HD],
                            in0=ps[:, :], scalar1=1.0 / WS, scalar2=None,
                            op0=ALU.mult)
                        if pv == 33:
                            nc.vector.tensor_scalar(
                                out=vt[32:33, half * 8:(half + 1) * 8, 0:HD],
                                in0=ps[:, :], scalar1=1.0 / WS, scalar2=None,
                                op0=ALU.mult)
                    vb.append(vt)
                vna.append(vb)
            esB.close()
            esA_HT.close()  # HT8 dead

            # ---------- stage C+D: attention (pipelined heads), out_proj,
            # residual, LN2, h2^T; interleaved per batch elem ----------
            esCD = ExitStack()
            probsTp = esCD.enter_context(tc.tile_pool(name="probsT", bufs=8))
            rcp = esCD.enter_context(tc.tile_pool(name="rcp", bufs=3))
            rcbp = esCD.enter_context(tc.tile_pool(name="rcb", bufs=3))
            atT8p = esCD.enter_context(tc.tile_pool(name="atT8", bufs=2))
            aoTp = esCD.enter_context(tc.tile_pool(name="aoT", bufs=2 * DC))
            pwD = esCD.enter_context(tc.tile_pool(name="pwD", bufs=1))
            esDE = ExitStack()
            H2T8p = esDE.enter_context(
                tc.tile_pool(name="H2T8", bufs=NB, side="right"))
            x1p = esDE.enter_context(
                tc.tile_pool(name="x1", bufs=NB * 3, side="right"))
            esD3 = ExitStack()
            # bufs=7: h2nat(b) is consumed by emit_h2t(b), deferred one b
            # later, so two b's worth of LN tiles must stay alive
            h2natp = esD3.enter_context(
                tc.tile_pool(name="h2nat", bufs=7, side="right"))

            owt = pwD.tile([128, DC, D], F8, name="pwO", tag="pwO")
            nc.sync.dma_start(owt[:], ow_d[:, :, :])

            attnT8 = [None] * NB
            aoT = [[] for _ in range(NB)]
            x1 = [[None] * 3 for _ in range(NB)]
            h2nat = [None] * NB
            H2T8 = [None] * NB

            def emit_outproj_part(b, mcs):
                aoT[b].extend(project_dr(owt, attnT8[b], ob_sb, 1.0 / WS,
                                         aoTp, "aoT", mcs=mcs))

            def emit_attention(b):
                attnT8[b] = atT8p.tile([128, DC, SP], F8, name="atT8",
                                       tag="atT8")
                pend = [None] * H       # fused [128, 2, SE] exp tile per head
                pend2 = [None] * H      # j2 exp per head pair (or per head)

                def emit_attn_mm(h):
                    dc, po = h // 2, (h % 2) * 64
                    r2 = (h % 2) * 32 if J2PACK else 0
                    at = pp.tile([HD + 1, SE], F32, name="psA", tag="pp")
                    pj0 = pend[h][:, 0, 0:SE] if FUSE_EXP else pend[h][0][:]
                    pj1 = pend[h][:, 1, 0:SE] if FUSE_EXP else pend[h][1][:]
                    nc.tensor.matmul(
                        at[:], vna[b][0][:, h, :], pj0,
                        start=True, stop=False)
                    nc.tensor.matmul(
                        at[:], vna[b][1][:, h, :], pj1,
                        start=False, stop=False)
                    nc.tensor.matmul(
                        at[:], vna[b][2][r2:r2 + 1, h, :],
                        pend2[dc if J2PACK else h][r2:r2 + 1, :],
                        start=False, stop=True)
                    # csum row must move to SBUF before reciprocal: the
                    # custom-DVE op mishandles psum base-partition offsets
                    cs = rcp.tile([1, SE], F32, name="cs", tag="cs")
                    nc.vector.tensor_copy(cs[:], at[HD:HD + 1, :])
                    rc = rcp.tile([1, SE], F32, name="rc", tag="rc")
                    nc.vector.reciprocal_approx_fast(rc[:], cs[:])
                    rcb = rcbp.tile([HD, SE], F32, name="rcb", tag="rcb")
                    nc.gpsimd.partition_broadcast(rcb[:], rc[:])
                    nc.vector.tensor_tensor(
                        out=attnT8[b][po:po + HD, dc, 0:SE],
                        in0=at[0:HD, :], in1=rcb[:], op=ALU.mult)

                for h in range(H):
                    dc, po = h // 2, (h % 2) * 64
                    if J2PACK and po == 0:
                        # last-key scores for the head pair at partitions 0/32
                        scj2 = pp.tile([33, SE], F32, name="psJ2", tag="pp")
                        nc.tensor.matmul(
                            scj2[0:1, :], kT[b][dc][0:64, 256:257],
                            qT[b][dc][0:64, :], start=True, stop=True)
                        nc.tensor.matmul(
                            scj2[32:33, :], kT[b][dc][64:128, 256:257],
                            qT[b][dc][64:128, :], start=True, stop=True)
                        pT2 = probsTp.tile([33, SE], F16, name="pT2",
                                           tag="pT2")
                        nc.scalar.activation(pT2[:], scj2[:], AF.Exp)
                        pend2[dc] = pT2
                    if not J2PACK:
                        scj2 = pp.tile([1, SE], F32, name="psJ2", tag="pp")
                        nc.tensor.matmul(
                            scj2[:], kT[b][dc][po:po + 64, 256:257],
                            qT[b][dc][po:po + 64, :], start=True, stop=True)
                        pT2 = probsTp.tile([1, SE], F16, name="pT2",
                                           tag="pT2")
                        nc.scalar.activation(pT2[:], scj2[:], AF.Exp)
                        pend2[h] = pT2
                    if FUSE_EXP:
                        scp = pq.tile([128, 2, 512], F32, name="psS",
                                      tag="pq")
                        for sj in range(2):
                            so = sj * 128
                            nc.tensor.matmul(
                                scp[:, sj, 0:SE],
                                kT[b][dc][po:po + 64, so:so + 128],
                                qT[b][dc][po:po + 64, :],
                                start=True, stop=True)
                        pTd = probsTp.tile([128, 2, SP], F16, name="pTd",
                                           tag="pTd")
                        nc.scalar.activation(pTd[:, :, 0:SE],
                                             scp[:, :, 0:SE], AF.Exp)
                        pend[h] = pTd
                    else:
                        pTs = []
                        for sj in range(2):
                            so = sj * 128
                            scT = pp.tile([128, SE], F32, name="psS",
                                          tag="pp")
                            nc.tensor.matmul(
                                scT[:],
                                kT[b][dc][po:po + 64, so:so + 128],
                                qT[b][dc][po:po + 64, :],
                                start=True, stop=True)
                            pT = probsTp.tile([128, SE], F16, name="pTd",
                                              tag="pTd")
                            nc.scalar.activation(pT[:], scT[:], AF.Exp)
                            pTs.append(pT)
                        pend[h] = pTs
                    if h > 1:
                        emit_attn_mm(h - 2)
                    if INTERLEAVE and h == 8 and b > 0:
                        emit_outproj_part(b - 1, range(0, 4))
                if INTERLEAVE and b > 0:
                    emit_outproj_part(b - 1, range(4, DC))
                emit_attn_mm(H - 2)
                emit_attn_mm(H - 1)

            def emit_natT_ln2(b):
                x1ts = []
                for j, (o, pz) in enumerate(SQ):
                    xres = xio.tile([pz, D], F32, name="xres", tag="xin")
                    nc.sync.dma_start(xres[:], x_d[b, o:o + pz, :])
                    x1t = x1p.tile([pz, D], F32, name="x1", tag="x1")
                    for hf in range(2):
                        ps = pp.tile([pz, 512], F16, name="psN", tag="pp")
                        for dl in range(4):
                            dc = hf * 4 + dl
                            nc.tensor.transpose(
                                ps[:, dl * 128:(dl + 1) * 128],
                                aoT[b][dc][:, o:o + pz], ident16[:128, :128])
                        nc.vector.tensor_tensor(
                            out=x1t[:, hf * 512:(hf + 1) * 512], in0=ps[:],
                            in1=xres[:, hf * 512:(hf + 1) * 512], op=ALU.add)
                    x1ts.append(x1t)
                x1[b] = x1ts
                h2nat[b] = layer_norm(x1ts, h2natp)

            def emit_h2t(b):
                t8 = H2T8p.tile([128, DC, SP], F8, name="H2T8", tag="H2T8")
                transpose_to_T8(h2nat[b], t8, g2_sb, b2_sb)
                H2T8[b] = t8

            if INTERLEAVE:
                for b in range(NB):
                    emit_attention(b)  # interleaves out_proj(b-1)
                    if b > 0:
                        emit_natT_ln2(b - 1)
                    if b > 1:
                        emit_h2t(b - 2)
                emit_outproj_part(NB - 1, range(DC))
                emit_natT_ln2(NB - 1)
                emit_h2t(NB - 2)
                emit_h2t(NB - 1)
            else:
                for b in range(NB):
                    emit_attention(b)
                    emit_outproj_part(b, range(DC))
                    emit_natT_ln2(b)
                    if b > 0:
                        emit_h2t(b - 1)
                emit_h2t(NB - 1)
            esCD.close()
            esBC.close()
            esD3.close()

            # ---------- stage E: MLP, single weight pass over all b ----------
            esE = ExitStack()
            w1p = esE.enter_context(tc.tile_pool(name="w1", bufs=3))
            w2p = esE.enter_context(tc.tile_pool(name="w2", bufs=2))
            h1Tp = esE.enter_context(tc.tile_pool(name="h1T", bufs=NB))
            moTp = esE.enter_context(tc.tile_pool(name="moT", bufs=NB * DC))
            outnp = esE.enter_context(tc.tile_pool(name="outn", bufs=3))

            h1T8 = [h1Tp.tile([128, FC, SP], F8, name="h1T8", tag="h1T8")
                    for b in range(NB)]
            for mc in range(FC):
                w1t = w1p.tile([128, DC, 128], F8, name="w1", tag="w1")
                nc.sync.dma_start(w1t[:], f1w_d[mc])
                for b in range(NB):
                    ps = pp.tile([128, SE], F32, name="psF1", tag="pp")
                    for kp in range(KP):
                        nc.tensor.matmul(
                            ps[:], w1t[:, 2 * kp:2 * kp + 2, :],
                            H2T8[b][:, 2 * kp:2 * kp + 2, 0:SE],
                            start=(kp == 0), stop=(kp == KP - 1),
                            perf_mode=DR)
                    nc.scalar.activation(h1T8[b][:, mc, 0:SE], ps[:],
                                         AF.Gelu_apprx_sigmoid,
                                         bias=f1b_sb[:, mc:mc + 1],
                                         scale=1.0 / WS)
            moT = [[None] * DC for _ in range(NB)]
            for mc in range(DC):
                w2t = w2p.tile([128, FC, 128], F8, name="w2", tag="w2")
                nc.sync.dma_start(w2t[:], f2w_d[mc])
                for b in range(NB):
                    ps = pp.tile([128, SE], F32, name="psF2", tag="pp")
                    for kp in range(FC // 2):
                        nc.tensor.matmul(
                            ps[:], w2t[:, 2 * kp:2 * kp + 2, :],
                            h1T8[b][:, 2 * kp:2 * kp + 2, 0:SE],
                            start=(kp == 0), stop=(kp == FC // 2 - 1),
                            perf_mode=DR)
                    t = moTp.tile([128, SE], F16, name="moT", tag="moT")
                    nc.vector.tensor_scalar(
                        out=t[:], in0=ps[:], scalar1=1.0 / WS,
                        scalar2=f2b_sb[:, mc:mc + 1],
                        op0=ALU.mult, op1=ALU.add)
                    moT[b][mc] = t
            for b in range(NB):
                for j, (o, pz) in enumerate(SQ):
                    ot = outnp.tile([pz, D], F32, name="outn", tag="outn")
                    for hf in range(2):
                        ps = pp.tile([pz, 512], F16, name="psO", tag="pp")
                        for dl in range(4):
                            dc = hf * 4 + dl
                            nc.tensor.transpose(
                                ps[:, dl * 128:(dl + 1) * 128],
                                moT[b][dc][:, o:o + pz], ident16[:128, :128])
                        nc.vector.tensor_tensor(
                            out=ot[:, hf * 512:(hf + 1) * 512], in0=ps[:],
                            in1=x1[b][j][:, hf * 512:(hf + 1) * 512],
                            op=ALU.add)
                    nc.sync.dma_start(out_d[b, o:o + pz, :], ot[:])
            esE.close()
            esDE.close()

    nc.compile()
    return nc


_NC = None


def _get_nc():
    global _NC
    if _NC is None:
        _NC = build()
    return _NC


def _q8(w, scale):
    """Quantize to TRN fp8e4 (e4m3, +-240) with a power-of-2 scale."""
    import ml_dtypes
    q = np.clip(w * scale, -240.0, 240.0).astype(ml_dtypes.float8_e4m3fn)
    return q.view(np.uint8)


def _pack_dd(w8):
    """[D, M] (wT layout, quantized) -> [128, DC, M] p-major tile."""
    return np.ascontiguousarray(
        w8.reshape(DC, 128, w8.shape[1]).transpose(1, 0, 2))


def _prep_inputs(inputs):
    f = lambda a: np.ascontiguousarray(np.asarray(a, dtype=np.float32))
    x = f(inputs["hidden_states"])
    qw8 = _q8(f(inputs["q_w"]).T * SCALE, QS)
    kw8 = _q8(f(inputs["k_w"]).T, WS)
    vw8 = _q8(f(inputs["v_w"]).T, WS)
    ow8 = _q8(f(inputs["o_w"]).T / AS, WS)
    f1w8 = _q8(f(inputs["fc1_w"]).T, WS)   # [D, FF]
    f2w8 = _q8(f(inputs["fc2_w"]).T, WS)   # [FF, D]
    shared = {
        "qw8": _pack_dd(qw8),
        "kw8": _pack_dd(kw8),
        "vw8": _pack_dd(vw8),
        "ow8": _pack_dd(ow8),
        # [D, FF] -> [FC, 128(p), DC(kc), 128(ml)]
        "f1w8": np.ascontiguousarray(
            f1w8.reshape(DC, 128, FC, 128).transpose(2, 1, 0, 3)),
        # [FF, D] -> [DC, 128(p), FC(kc), 128(ml)]
        "f2w8": np.ascontiguousarray(
            f2w8.reshape(FC, 128, DC, 128).transpose(2, 1, 0, 3)),
        "qb": f(inputs["q_b"]) * SCALE,
        "kb": f(inputs["k_b"]),
        "ob": f(inputs["o_b"]) + f(inputs["o_w"]) @ f(inputs["v_b"]),
        "f1b": f(inputs["fc1_b"]),
        "f2b": f(inputs["fc2_b"]),
        "g1": f(inputs["ln1_g"]),
        "b1": f(inputs["ln1_b"]),
        "g2": f(inputs["ln2_g"]),
        "b2": f(inputs["ln2_b"]),
    }
    shared = {k: np.ascontiguousarray(v) for k, v in shared.items()}
    in_maps = []
    for c in range(N_CORES):
        m = dict(shared)
        m["x"] = np.ascontiguousarray(x[c * NB:(c + 1) * NB])
        in_maps.append(m)
    return in_maps


def run(inputs, trace=False):
    nc = _get_nc()
    in_maps = _prep_inputs(inputs)
    res = bass_utils.run_bass_kernel_spmd(
        nc, in_maps, core_ids=list(range(N_CORES)), trace=trace)
    out = np.concatenate([res.results[c]["out"] for c in range(N_CORES)],
                         axis=0)
    return out, res


def kernel(**inputs):
    out, _ = run(inputs, trace=False)
    return out
